# revision 18
# baseline (speedup 1.0000x reference)
"""Trainium2 Bass kernel for the BAN (bilinear attention network) problem.

Math (per batch b, eval mode):
    hq = emb[he_ques] @ Wq + bq                  [NQ, H]
    hk = emb[he_kg]   @ Wk + bk                  [NK, H]
    logits[g,q,k] = sum_d hq[q,d] Watt[d,g] hk[k,d]   (+ batt[g], cancels in
                                                       the joint softmax)
    att = softmax over flattened (q,k) per (b,g)
    pooled[g,d] = sum_{q,k} hq[q,d] att[g,q,k] hk[k,d]
    out = pooled.flat @ Wout + bout;  sim = out @ glove.T;  log_softmax(sim)

Distribution: pure data parallel over batch, 8 samples per core on 8 cores.
All weights replicated. No collectives.

v2 design notes (vs the f32r v1):
  - bf16 operands everywhere (PSUM accumulation stays f32). Host converts
    weights/emb to bf16; rel-err budget is 2e-2 so ~1e-3..1e-2 is fine.
  - ALL transposes ride the DMA engines (XBAR dma_start_transpose, 2-byte
    dtype): gathered emb rows -> e-major, hk -> hkT, out -> outT. Zero PE
    transposes, no transpose PSUM traffic. Every XBAR destination is a
    fully contiguous tile (sliced dsts produce wrong data on HW).
  - The K projection is computed once (token-major hk via matmul); the
    d-major hkT needed by the logits matmul is a DMA transpose of it.
  - PSUM accumulation groups are interleaved two-wide across banks so the
    ~0.3-0.5us "stop" drain of one group hides under the next group's
    matmuls. PSUM->SBUF copies alternate Vector/Scalar engines.
  - Weight streams are host-packed so each is a few big DMAs (contiguous
    per-partition runs); all issued up-front on the SP queue.
  - log-softmax tail: sim values are O(+-5), so exp needs no max pass.
    Each sim chunk exp+accumulates on ACT right after its matmuls while
    later chunks still stream; sim stays resident in PSUM (8 sub-tiles in
    2 banks via partition offsets); the final (sim - logZ) pass is split
    across Vector/Scalar/GpSimd and DMA'd out per chunk.
  - PE warmup starts on a memset (no DMA dependency) so the clock governor
    ramps while the first gathers/DMAs are still in flight.
"""

import sys

if "/opt/trn_rl_repo" not in sys.path:
    sys.path.insert(0, "/opt/trn_rl_repo")

import numpy as np

import concourse.bass as bass
import concourse.mybir as mybir
import concourse.tile as tile
from concourse import bacc
from concourse.bass_utils import run_bass_kernel_spmd

F32 = mybir.dt.float32
BF16 = mybir.dt.bfloat16
I32 = mybir.dt.int32
AX = mybir.AxisListType
OP = mybir.AluOpType
AF = mybir.ActivationFunctionType

N_CORES = 8
VOCAB = 20000
E = 300          # word embedding size
EA = 384         # padded: col 300 = ones (bias trick), 301..383 = 0
H = 1024         # hidden
G = 8            # heads
N_OUT = 300
N_ANS = 4000
B, NQ, NK = 64, 32, 256
BL = B // N_CORES            # 8 samples per core
TQ = BL * NQ                 # 256 q tokens per core
TK = BL * NK                 # 2048 k tokens per core
TQ_TILES = TQ // 128         # 2
TK_TILES = TK // 128         # 16
DT = H // 128                # 8 d-tiles
E_ROWS = (128, 128, 45)      # valid rows per e-chunk (301 used rows)
N_ROWS = (128, 128, 44)      # valid rows per N_OUT chunk (300 rows)
NA_CH = 8                    # sim computed in 8 chunks of 500
NA_W = N_ANS // NA_CH        # 500
N_WARM = 12                  # PE warmup matmuls (512 cols each)


def build_kernel():
    nc = bacc.Bacc("TRN2", target_bir_lowering=False, debug=False,
                   num_devices=N_CORES)

    # ---- DRAM I/O ----
    emb_d = nc.dram_tensor("emb", [VOCAB, EA], BF16, kind="ExternalInput").ap()
    idxq_d = nc.dram_tensor("idx_q", [128, TQ_TILES], I32, kind="ExternalInput").ap()
    idxk_d = nc.dram_tensor("idx_k", [128, TK_TILES], I32, kind="ExternalInput").ap()
    # host-packed weight layouts (see make_in_maps)
    wq_d = nc.dram_tensor("wq", [128, 3, H], BF16, kind="ExternalInput").ap()
    wk_d = nc.dram_tensor("wk", [128, 3, H], BF16, kind="ExternalInput").ap()
    wattx_d = nc.dram_tensor("wattx", [128, DT, G, NQ], BF16,
                             kind="ExternalInput").ap()
    wout_d = nc.dram_tensor("wout", [8, 128, G, N_OUT], BF16,
                            kind="ExternalInput").ap()
    glove_d = nc.dram_tensor("glove", [NA_CH, 128, 3, NA_W], BF16,
                             kind="ExternalInput").ap()
    bout_d = nc.dram_tensor("bout", [BL, N_OUT], F32, kind="ExternalInput").ap()
    ones_d = nc.dram_tensor("ones_col", [128, 1], BF16, kind="ExternalInput").ap()
    out_d = nc.dram_tensor("out", [BL, N_ANS], F32, kind="ExternalOutput").ap()
    warm_d = nc.dram_tensor("warm", [1, 128], F32, kind="ExternalOutput").ap()

    with tile.TileContext(nc) as tc:
        import contextlib

        with contextlib.ExitStack() as ctx:
            consts = ctx.enter_context(tc.tile_pool(name="consts", bufs=1))
            xrk_p = ctx.enter_context(tc.tile_pool(name="xrk", bufs=4))
            xkt_p = ctx.enter_context(tc.tile_pool(name="xkt", bufs=4))
            hk_p = ctx.enter_context(tc.tile_pool(name="hk", bufs=16))
            hkt_p = ctx.enter_context(tc.tile_pool(name="hkt", bufs=16))
            hqw_p = ctx.enter_context(tc.tile_pool(name="hqw", bufs=4))
            et_p = ctx.enter_context(tc.tile_pool(name="et", bufs=2))
            zz_p = ctx.enter_context(tc.tile_pool(name="zz", bufs=2))
            zn_p = ctx.enter_context(tc.tile_pool(name="zn", bufs=2))
            v_p = ctx.enter_context(tc.tile_pool(name="v", bufs=2))
            mm_p = ctx.enter_context(tc.tile_pool(name="mm", bufs=4, space="PSUM"))
            lg_p = ctx.enter_context(tc.tile_pool(name="lg", bufs=2, space="PSUM"))
            up_p = ctx.enter_context(tc.tile_pool(name="up", bufs=2, space="PSUM"))

            def copy_on(idx, out, in_):
                """PSUM->SBUF copy on alternating engines (0=DVE, 1=ACT)."""
                if idx % 2 == 0:
                    nc.vector.tensor_copy(out, in_)
                else:
                    nc.scalar.activation(out=out, in_=in_, func=AF.Copy)

            # ---- PE warmup: no DMA dependency (memset source tiles) ----
            warm_a = consts.tile([128, 128], BF16, tag="warm_a")
            warm_b = consts.tile([128, 512], BF16, tag="warm_b")
            nc.gpsimd.memset(warm_a[:], 1.0)
            nc.gpsimd.memset(warm_b[:], 0.5)
            wps = mm_p.tile([128, 512], F32, tag="mm")
            for i in range(N_WARM):
                nc.tensor.matmul(wps[:], lhsT=warm_a[:], rhs=warm_b[:],
                                 start=(i == 0), stop=(i == N_WARM - 1))
            warm_sb = consts.tile([1, 128], F32, tag="warm")
            nc.vector.tensor_copy(warm_sb[:], wps[:1, :128])
            nc.sync.dma_start(warm_d, warm_sb[:])

            # ---- weight/index DMAs, all up-front on SP queue ----
            idxq_sb = consts.tile([128, TQ_TILES], I32, tag="idxq")
            nc.sync.dma_start(idxq_sb[:], idxq_d)
            idxk_sb = consts.tile([128, TK_TILES], I32, tag="idxk")
            nc.sync.dma_start(idxk_sb[:], idxk_d)
            wq_sb = consts.tile([128, 3, H], BF16, tag="wq")
            nc.sync.dma_start(wq_sb[:], wq_d)
            wk_sb = consts.tile([128, 3, H], BF16, tag="wk")
            nc.sync.dma_start(wk_sb[:], wk_d)
            wattx_sb = consts.tile([128, DT, G, NQ], BF16, tag="wattx")
            nc.sync.dma_start(wattx_sb[:], wattx_d)
            bout_sb = consts.tile([BL, N_OUT], F32, tag="bout")
            nc.sync.dma_start(bout_sb[:], bout_d)
            ones_sb = consts.tile([128, 1], BF16, tag="ones")
            nc.sync.dma_start(ones_sb[:], ones_d)

            # ---- indirect gathers (Pool queue), all up-front ----
            xrow_q = consts.tile([128, TQ_TILES, EA], BF16, tag="xrow_q")
            for t in range(TQ_TILES):
                nc.gpsimd.indirect_dma_start(
                    out=xrow_q[:, t, :],
                    out_offset=None,
                    in_=emb_d,
                    in_offset=bass.IndirectOffsetOnAxis(
                        ap=idxq_sb[:, t : t + 1], axis=0
                    ),
                )
            xrow_k = []
            for p in range(4):
                xr = xrk_p.tile([128, 4, EA], BF16, tag="xrk")
                for t in range(4):
                    nc.gpsimd.indirect_dma_start(
                        out=xr[:, t, :],
                        out_offset=None,
                        in_=emb_d,
                        in_offset=bass.IndirectOffsetOnAxis(
                            ap=idxk_sb[:, 4 * p + t : 4 * p + t + 1], axis=0
                        ),
                    )
                xrow_k.append(xr)

            # ---- XBAR transposes of the gathered rows ----
            # in [128 tok, T*384] -> out [128 e, T*3, 128 tok]; the (t,c)
            # chunk index is t*3+c. xqT goes on the ACT queue (waits only
            # the first two gathers); xkT on SP.
            xqT = consts.tile([128, TQ_TILES * 3, 128], BF16, tag="xqT")
            nc.scalar.dma_start_transpose(xqT[:], xrow_q[:, :, :])
            xkT = []
            for p in range(4):
                xk = xkt_p.tile([128, 12, 128], BF16, tag="xkT")
                nc.sync.dma_start_transpose(xk[:], xrow_k[p][:, :, :])
                xkT.append(xk)

            # ---- ACT table preloads (Exp/Ln/Identity) off the critical path
            dum = consts.tile([1, 2], F32, tag="dum")
            nc.vector.tensor_copy(dum[:], wps[:1, :2])
            nc.scalar.activation(out=dum[:, :1], in_=dum[:, 1:], func=AF.Exp)
            nc.scalar.activation(out=dum[:, :1], in_=dum[:, 1:], func=AF.Ln)
            nc.scalar.activation(out=dum[:, :1], in_=dum[:, 1:], func=AF.Identity,
                                 bias=dum[:, 1:])

            # ---- phase C: hqT [128, DT, TQ] bf16 ----
            # rhs = xqT[:rows, (t,c), :] strided -> [rows, 256]
            hqT = consts.tile([128, DT, TQ], BF16, tag="hqT")
            cps = [None, None]
            for m in range(DT):
                s = m % 2
                if cps[s] is not None:
                    # copy previous group on alternating engines
                    mprev, ps = cps[s]
                    copy_on(mprev, hqT[:, mprev, :], ps[:, :TQ])
                ps = mm_p.tile([128, 512], F32, tag="mm")
                for c, rows in enumerate(E_ROWS):
                    nc.tensor.matmul(
                        ps[:, :TQ],
                        lhsT=wq_sb[:rows, c, m * 128 : (m + 1) * 128],
                        rhs=xqT[:rows, :, :].rearrange(
                            "p (t c) i -> p c t i", c=3
                        )[:, c, :, :],
                        start=(c == 0),
                        stop=(c == 2),
                    )
                cps[s] = (m, ps)
            for s in range(2):
                mprev, ps = cps[s]
                copy_on(mprev, hqT[:, mprev, :], ps[:, :TQ])

            poT = consts.tile([128, DT, G, BL], BF16, tag="poT")

            # ---- phase D0: ALL K projections + hkT transposes up-front ----
            # Dense 20us PE stream; the XBAR transposes' issue+DMA latency
            # hides behind it, so the attention loop has no DMA deps.
            hk_tiles = [None] * TK_TILES
            hkT_tiles = [None] * TK_TILES
            for kt in range(TK_TILES):
                p, t = kt // 4, kt % 4
                hk = hk_p.tile([128, H], BF16, tag="hk")
                pss = [mm_p.tile([128, 512], F32, tag="mm", name="pss") for _ in range(2)]
                # groups must be sequential: interleaved accumulation
                # groups at the same tile_position corrupt PSUM on HW
                for dh in range(2):
                    for c, rows in enumerate(E_ROWS):
                        nc.tensor.matmul(
                            pss[dh][:],
                            lhsT=xkT[p][:rows, t * 3 + c, :],
                            rhs=wk_sb[:rows, c, dh * 512 : (dh + 1) * 512],
                            start=(c == 0),
                            stop=(c == 2),
                        )
                for dh in range(2):
                    copy_on(kt + dh, hk[:, dh * 512 : (dh + 1) * 512], pss[dh][:])
                hk_tiles[kt] = hk
                # hkT via XBAR: [128 tok, 1024 d] -> [128 d, 8 m, 128 tok];
                # alternate issue queues (ACT / SP)
                hkT = hkt_p.tile([128, DT, 128], BF16, tag="hkT")
                if kt % 2 == 0:
                    nc.scalar.dma_start_transpose(hkT[:], hk[:, :])
                else:
                    nc.sync.dma_start_transpose(hkT[:], hk[:, :])
                hkT_tiles[kt] = hkT

            # ---- big weight streams (SP queue, after all transposes) ----
            glove_tiles = []
            for a in range(NA_CH):
                gt = consts.tile([128, 3, NA_W], BF16, tag=f"glove{a}")
                nc.sync.dma_start(gt[:], glove_d[a])
                glove_tiles.append(gt)
            wout_tiles = []
            for j in range(8):
                wt = consts.tile([128, G, N_OUT], BF16, tag=f"wout{j}")
                nc.sync.dma_start(wt[:], wout_d[j])
                wout_tiles.append(wt)

            # ---- attention loop: pure compute, no DMA dependencies ----
            for p in range(4):
                hqw_s = []
                for bi in range(2):
                    b = p * 2 + bi
                    hqw = hqw_p.tile([128, DT, G, NQ], BF16, tag="hqw")
                    nc.vector.tensor_tensor(
                        out=hqw[:],
                        in0=hqT[:, :, None, b * NQ : (b + 1) * NQ].to_broadcast(
                            [128, DT, G, NQ]
                        ),
                        in1=wattx_sb[:],
                        op=OP.mult,
                    )
                    hqw_s.append(hqw)
                for bi in range(2):
                    b = p * 2 + bi
                    hqw = hqw_s[bi]

                    # D5: logits.T [k, (g,q)] in PSUM [128, 2, 256]
                    ps_l = lg_p.tile([128, 512], F32, tag="lg")
                    for kt in range(2):
                        hkt = hkT_tiles[4 * p + 2 * bi + kt]
                        for c in range(DT):
                            nc.tensor.matmul(
                                ps_l[:, kt * 256 : (kt + 1) * 256],
                                lhsT=hkt[:, c, :],
                                rhs=hqw[:, c, :, :],
                                start=(c == 0),
                                stop=(c == DT - 1),
                            )

                    # D6: E = exp(logits) (no max needed: logits are O(+-6));
                    # per-(kt,g) row sums zz
                    et = et_p.tile([128, 2, G * NQ], BF16, tag="et")
                    zz = zz_p.tile([128, 2, G], BF16, tag="zz")
                    for kt in range(2):
                        nc.scalar.activation(
                            out=et[:, kt, :],
                            in_=ps_l[:, kt * 256 : (kt + 1) * 256],
                            func=AF.Exp,
                        )
                        with nc.allow_low_precision(reason="bf16 round of sum"):
                            nc.vector.tensor_reduce(
                                out=zz[:, kt, :],
                                in_=et[:, kt].rearrange("p (g q) -> p g q", g=G),
                                axis=AX.X,
                                op=OP.add,
                            )

                    # D7: Z_g = sum over k-partitions via ones matmul;
                    # zinv broadcast to all partitions on GpSimd
                    ps_z = mm_p.tile([128, 512], F32, tag="mm")
                    nc.tensor.matmul(
                        ps_z[:1, : 2 * G],
                        lhsT=ones_sb[:],
                        rhs=zz[:],
                        start=True,
                        stop=True,
                    )
                    z2 = zn_p.tile([1, G], F32, tag="z2")
                    nc.vector.tensor_reduce(
                        out=z2[:],
                        in_=ps_z[:1, : 2 * G].rearrange("p (kt g) -> p g kt", g=G),
                        axis=AX.X,
                        op=OP.add,
                    )
                    zinv = zn_p.tile([1, G], F32, tag="zinv")
                    nc.vector.reciprocal(zinv[:1, :], z2[:1, :])
                    zbro = zn_p.tile([128, G], F32, tag="zbro")
                    nc.gpsimd.partition_broadcast(zbro[:], zinv[:1, :], channels=128)

                    # D8: u.T, v, pooled partial sums; 2 d-tiles per PSUM tile
                    for mp in range(4):
                        ps_u = up_p.tile([128, 512], F32, tag="up")
                        # sequential groups per column range (interleaving
                        # corrupts PSUM accumulation on HW)
                        for mi in range(2):
                            m = mp * 2 + mi
                            for kt in range(2):
                                hkl = hk_tiles[4 * p + 2 * bi + kt]
                                nc.tensor.matmul(
                                    ps_u[:, mi * 256 : (mi + 1) * 256],
                                    lhsT=hkl[:, m * 128 : (m + 1) * 128],
                                    rhs=et[:, kt, :],
                                    start=(kt == 0),
                                    stop=(kt == 1),
                                )
                        v = v_p.tile([128, 2, G, NQ], BF16, tag="v")
                        nc.vector.tensor_tensor(
                            out=v[:],
                            in0=ps_u[:].rearrange("p (m g q) -> p m g q", m=2, g=G),
                            in1=hqT[
                                :, mp * 2 : mp * 2 + 2, None, b * NQ : (b + 1) * NQ
                            ].to_broadcast([128, 2, G, NQ]),
                            op=OP.mult,
                        )
                        vr = v_p.tile([128, 2, G], BF16, tag="vr")
                        with nc.allow_low_precision(reason="bf16 round of sum"):
                            nc.vector.tensor_reduce(
                                out=vr[:], in_=v[:], axis=AX.X, op=OP.add
                            )
                            nc.vector.tensor_tensor(
                                out=poT[:, mp * 2 : mp * 2 + 2, :, b],
                                in0=vr[:],
                                in1=zbro[:, None, :].to_broadcast([128, 2, G]),
                                op=OP.mult,
                            )

            # ---- phase F: out [8, 300] = pooled_flat @ Wout + bout ----
            ps_o = mm_p.tile([128, 512], F32, tag="mm")
            for g in range(G):
                for m in range(DT):
                    t = g * DT + m
                    nc.tensor.matmul(
                        ps_o[:BL, :N_OUT],
                        lhsT=poT[:, m, g, :],
                        rhs=wout_tiles[t // 8][:, t % 8, :],
                        start=(t == 0),
                        stop=(t == G * DT - 1),
                    )
            # padded [16, 384] so XBAR can transpose it (rows%16, cols%128)
            out_sb = consts.tile([16, 3, 128], BF16, tag="out_sb")
            nc.gpsimd.memset(out_sb[:], 0.0)
            nc.vector.tensor_tensor(
                out=out_sb[:BL, :, :].rearrange("b c i -> b (c i)")[:, :N_OUT],
                in0=ps_o[:BL, :N_OUT],
                in1=bout_sb[:],
                op=OP.add,
            )
            outT = consts.tile([128, 3, 16], BF16, tag="outT")
            nc.scalar.dma_start_transpose(outT[:], out_sb[:, :, :])

            # ---- phase G: sim + log_softmax, psum-resident ----
            # 8 chunks of [8, 500] live in 3 PSUM banks at partition offsets
            # 0/32/64. Interleave chunk groups two-wide; exp+accumulate on
            # ACT right after each chunk's last matmul.
            ps_s = [mm_p.tile([128, 512], F32, tag="mm", name="ps_s") for _ in range(3)]
            esc = consts.tile([BL, NA_W], BF16, tag="esc")
            zs8 = consts.tile([BL, NA_CH], F32, tag="zs8")
            zs = consts.tile([BL, 1], F32, tag="zs")
            lnz = consts.tile([BL, 1], F32, tag="lnz")
            nlnz = consts.tile([BL, 1], F32, tag="nlnz")

            def sim_psum(a):
                off = 32 * (a % 3)
                return ps_s[a // 3][off : off + BL, :NA_W]

            for a0 in range(0, NA_CH, 2):
                for c, rows in enumerate(N_ROWS):
                    for a in (a0, a0 + 1):
                        nc.tensor.matmul(
                            sim_psum(a),
                            lhsT=outT[:rows, c, :BL],
                            rhs=glove_tiles[a][:rows, c, :],
                            start=(c == 0),
                            stop=(c == 2),
                            skip_group_check=True,
                        )
                for a in (a0, a0 + 1):
                    nc.scalar.activation(
                        out=esc[:],
                        in_=sim_psum(a),
                        func=AF.Exp,
                        accum_out=zs8[:, a : a + 1],
                    )
            nc.vector.tensor_reduce(out=zs[:], in_=zs8[:], axis=AX.X, op=OP.add)
            nc.scalar.activation(out=lnz[:], in_=zs[:], func=AF.Ln)
            nc.vector.tensor_scalar_mul(nlnz[:], lnz[:], -1.0)
            # final: res = sim - lnz, split across Vector/Scalar
            for a in range(NA_CH):
                dst = zz_p.tile([BL, NA_W], F32, tag="res", name="res")[:]
                if a % 2 == 0:
                    nc.vector.tensor_scalar(
                        out=dst, in0=sim_psum(a), scalar1=lnz[:], scalar2=None,
                        op0=OP.subtract,
                    )
                else:
                    nc.scalar.activation(
                        out=dst, in_=sim_psum(a), func=AF.Identity, bias=nlnz[:],
                    )
                nc.sync.dma_start(out_d[:, a * NA_W : (a + 1) * NA_W], dst)

    nc.compile()
    return nc


_NC = None


def _get_nc():
    global _NC
    if _NC is None:
        _NC = build_kernel()
    return _NC


def make_in_maps(inputs):
    import ml_dtypes
    bf16 = ml_dtypes.bfloat16

    he_q = np.asarray(inputs["he_ques"]).astype(np.int32)   # [64, 32]
    he_k = np.asarray(inputs["he_kg"]).astype(np.int32)     # [64, 256]
    emb0 = np.asarray(inputs["emb"], dtype=np.float32)
    emb = np.zeros((VOCAB, EA), dtype=np.float32)
    emb[:, :E] = emb0
    emb[:, E] = 1.0                                         # bias ones column
    emb = emb.astype(bf16)

    def pack_w(w, b):
        # [300, H] + bias -> padded [384, H] -> [128, 3, H]
        wp = np.zeros((EA, H), dtype=np.float32)
        wp[:E] = np.asarray(w, np.float32)
        wp[E] = np.asarray(b, np.float32)
        return np.ascontiguousarray(
            wp.reshape(3, 128, H).transpose(1, 0, 2)).astype(bf16)

    wq = pack_w(inputs["Wq"], inputs["bq"])
    wk = pack_w(inputs["Wk"], inputs["bk"])

    # watt expanded: [128, DT, G, NQ] (broadcast along the q-columns)
    watt = np.asarray(inputs["Watt"], np.float32).reshape(DT, 128, G)
    wattx = np.ascontiguousarray(
        np.broadcast_to(
            watt.transpose(1, 0, 2)[:, :, :, None], (128, DT, G, NQ)
        )
    ).astype(bf16)

    # wout packed: [8, 128, 8, 300]; tile t=g*DT+m lives at [t//8, :, t%8, :]
    wout = np.asarray(inputs["Wout"], np.float32).reshape(8, 8, 128, N_OUT)
    wout = np.ascontiguousarray(wout.transpose(0, 2, 1, 3)).astype(bf16)

    # glove packed: [NA_CH, 128, 3, 500] from padded glove.T [384, 4000]
    gt = np.zeros((EA, N_ANS), dtype=np.float32)
    gt[:N_OUT] = np.asarray(inputs["glove_cands"], np.float32).T
    glove = np.ascontiguousarray(
        gt.reshape(3, 128, NA_CH, NA_W).transpose(2, 1, 0, 3)).astype(bf16)

    bout = np.ascontiguousarray(
        np.broadcast_to(np.asarray(inputs["bout"], np.float32), (BL, N_OUT)))

    in_maps = []
    for i in range(N_CORES):
        iq = he_q[i * BL : (i + 1) * BL].reshape(-1)        # [256]
        ik = he_k[i * BL : (i + 1) * BL].reshape(-1)        # [2048]
        in_maps.append({
            "emb": emb,
            "idx_q": np.ascontiguousarray(iq.reshape(TQ_TILES, 128).T),
            "idx_k": np.ascontiguousarray(ik.reshape(TK_TILES, 128).T),
            "wq": wq,
            "wk": wk,
            "wattx": wattx,
            "wout": wout,
            "glove": glove,
            "bout": bout,
            "ones_col": np.ones((128, 1), dtype=np.float32).astype(bf16),
        })
    return in_maps


def kernel(**inputs) -> np.ndarray:
    nc = _get_nc()
    in_maps = make_in_maps(inputs)
    res = run_bass_kernel_spmd(nc, in_maps, list(range(N_CORES)))
    return np.concatenate([res.results[i]["out"] for i in range(N_CORES)], axis=0)
